# revision 12
# baseline (speedup 1.0000x reference)
"""Trainium2 Bass kernel for ExpanderLinearLayer (gather-mul-scatter_add).

Reformulation: out = input_ @ S + bias, where S[i, j] = sum of weight[k] over
all k with ind_in[k] == i and ind_out[k] == j.  S is built dense on the host
(52224 nnz into 1024x1024) and the device runs a dense bf16 matmul,
data-parallel over the batch across 8 NeuronCores.

Per core (batch shard of 512 rows): the 1024-long contraction dim is split
into 8 chunks of 128 partitions.  All device I/O is bf16 (errors ~2e-3 vs
the 2e-2 gate): input DMA is 3.0 MB instead of 6.3, output 1 MB instead of 2.

  chunk 0 is split into two DMAs ([x_0 | s_0 m0,m1] and [s_0 m2..7]) so the
  first matmul can start ~0.5us after data starts flowing.
  chunk k (k>=1):  [x_k | s_k]  at cols [k*1536, (k+1)*1536)
      x_k[p, n] = input_[c*512+n, k*128+p]   (n < 512)
      s_k[p, m*128+q] = S[k*128+p, m*128+q]
  bias arrives as a separate tiny f32 tensor [128, 8] on the scalar ring.

Matmul (k outer, m inner): psum[m] += s_km.T @ x_k, accumulated over k in 8
PSUM banks.  A few junk "prewarm" matmuls run during the DMA fill so the PE
HAM clock-gate (1.2 GHz cold -> 2.4 GHz warm after ~3.4us of activity) is
released before the real stream starts.

Epilogue streams: as each psum bank finishes (k=7), bias-add + bf16 cast on
Vector (even m) / Scalar-ACT Identity (odd m), then paired output DMAs on
the two HWDGE rings so the tail after the last matmul is short.
"""

import os
import numpy as np
import ml_dtypes

try:
    from concourse import bacc, bass, mybir
    from concourse.tile import TileContext
    from concourse.bass_utils import run_bass_kernel_spmd
except ImportError:  # fresh dir without PYTHONPATH
    import sys

    sys.path.insert(0, "/opt/trn_rl_repo")
    from concourse import bacc, bass, mybir
    from concourse.tile import TileContext
    from concourse.bass_utils import run_bass_kernel_spmd

P = 128
B = 4096
D = 1024
NCORES = 8
BS = B // NCORES      # 512 batch rows per core
KO = D // P           # 8 contraction chunks
MO = D // P           # 8 output tiles
CW = BS + D           # 1536 columns per merged chunk
NWARM = 25            # junk matmuls to pre-warm the PE HAM clock gate
KSPLIT = 4            # chunks 0..KSPLIT-1 run k-outer; rest run bank-major

F32 = mybir.dt.float32
BF16 = mybir.dt.bfloat16
BF16_NP = ml_dtypes.bfloat16

_NC_CACHE = {}
LAST_RESULTS = None


def _build_nc():
    nc = bacc.Bacc("TRN2", target_bir_lowering=False)
    xs_d = nc.declare_dram_parameter("xs", [P, KO * CW], BF16, isOutput=False)
    bs_d = nc.declare_dram_parameter("bs", [P, MO], F32, isOutput=False)
    o_d = nc.declare_dram_parameter("o", [P, MO, BS], F32, isOutput=True)

    with TileContext(nc) as tc:
        with (
            tc.tile_pool(name="cs", bufs=1) as cpool,
            tc.tile_pool(name="bb", bufs=1) as bpool,
            tc.tile_pool(name="ob", bufs=1) as opool,
            tc.tile_pool(name="wj", bufs=1) as wpool,
            tc.tile_pool(name="ps", bufs=1, space="PSUM") as pspool,
        ):
            psums = [
                pspool.tile([P, BS], F32, tag=f"ps{m}", name=f"ps{m}")
                for m in range(MO)
            ]

            # PE prewarm: junk matmuls with no DMA dependency keep the PE
            # busy during the DMA fill so HAM releases the clock gate
            # before the real stream starts.  N=128 keeps the granularity
            # fine so the real stream isn't delayed when chunk 0 lands.
            junk = wpool.tile([P, P], BF16, tag="junk", name="junk")
            nc.gpsimd.memset(junk, 0.0)
            for w in range(NWARM):
                nc.tensor.matmul(
                    psums[0][:, :P], lhsT=junk, rhs=junk, start=True, stop=True
                )

            bias_sb = bpool.tile([P, MO], F32, tag="bias", name="bias")
            nc.scalar.dma_start(bias_sb, bs_d[:, :])

            # Input chunks on the sync HWDGE ring, in consumption order.
            # Chunk 0 is split (fast first matmul); later chunks are merged
            # into fewer, larger DMAs — each chunk-boundary semaphore wait
            # costs the PE ~0.4-0.6us, and arrival granularity no longer
            # matters once the stream is ahead of the DMA.
            c0a = cpool.tile([P, BS + 2 * P], BF16, tag="c0a", name="c0a")
            c0b = cpool.tile([P, 6 * P], BF16, tag="c0b", name="c0b")
            nc.sync.dma_start(c0a, xs_d[:, : BS + 2 * P])
            nc.sync.dma_start(c0b, xs_d[:, BS + 2 * P : CW])
            groups = [(1, 2), (3, 4), (5, 6), (7,)]
            gtiles = {}
            for g in groups:
                k0, k1 = g[0], g[-1]
                gt = cpool.tile(
                    [P, len(g) * CW], BF16, tag=f"c{k0}{k1}", name=f"c{k0}{k1}"
                )
                nc.sync.dma_start(gt, xs_d[:, k0 * CW : (k1 + 1) * CW])
                for k in g:
                    gtiles[k] = (gt, (k - k0) * CW)

            def chunk_x(k):
                if k == 0:
                    return c0a[:, :BS]
                gt, off = gtiles[k]
                return gt[:, off : off + BS]

            def chunk_s(k, m):
                if k == 0:
                    if m < 2:
                        return c0a[:, BS + m * P : BS + (m + 1) * P]
                    return c0b[:, (m - 2) * P : (m - 1) * P]
                gt, off = gtiles[k]
                return gt[:, off + BS + m * P : off + BS + (m + 1) * P]

            # Phase A (k-outer, m-inner): chunks 0..KSPLIT-1 consumed in
            # DMA-arrival order while later chunks stream in.
            for k in range(KSPLIT):
                rhs = chunk_x(k)
                for m in range(MO):
                    nc.tensor.matmul(
                        psums[m],
                        lhsT=chunk_s(k, m),
                        rhs=rhs,
                        start=(k == 0),
                        stop=False,
                    )

            # Phase B (bank-major): by the time the PE gets here, all
            # chunks are (nearly) resident, so finish one psum bank at a
            # time.  Bank m completes ~0.9us apart, letting the bias-add
            # and its output DMA stream underneath the matmuls instead of
            # piling up after the last one.
            out_sb = opool.tile([P, MO, BS], F32, tag="out", name="out")
            for m in range(MO):
                for k in range(KSPLIT, KO):
                    nc.tensor.matmul(
                        psums[m],
                        lhsT=chunk_s(k, m),
                        rhs=chunk_x(k),
                        start=False,
                        stop=(k == KO - 1),
                    )
                if m % 2 == 0:
                    nc.vector.tensor_scalar_add(
                        out_sb[:, m], psums[m], bias_sb[:, m : m + 1]
                    )
                else:
                    nc.scalar.activation(
                        out_sb[:, m],
                        psums[m],
                        mybir.ActivationFunctionType.Identity,
                        bias=bias_sb[:, m : m + 1],
                    )
                # Last bank's DMA goes on the scalar ring: the sync ring may
                # still be busy issuing bank 6's, and bank 7's chain is the
                # kernel's critical tail.
                eng = nc.scalar if m == MO - 1 else nc.sync
                eng.dma_start(o_d[:, m], out_sb[:, m])

    nc.finalize()
    return nc


def _get_nc():
    if "nc" not in _NC_CACHE:
        _NC_CACHE["nc"] = _build_nc()
    return _NC_CACHE["nc"]


def kernel(input_, weight, bias, ind_in, ind_out):
    global LAST_RESULTS
    input_ = np.asarray(input_, dtype=np.float32)
    weight = np.asarray(weight, dtype=np.float32)
    bias = np.asarray(bias, dtype=np.float32)
    ind_in = np.asarray(ind_in, dtype=np.int64)
    ind_out = np.asarray(ind_out, dtype=np.int64)

    # Dense scatter matrix S (f32 accumulate, then bf16).
    S = np.zeros((D, D), np.float32)
    np.add.at(S, (ind_in, ind_out), weight)
    Sb = S.astype(BF16_NP)
    b_l = np.ascontiguousarray(bias.reshape(MO, P).T)  # [128, 8] f32

    in_maps = []
    for c in range(NCORES):
        xT = input_[c * BS : (c + 1) * BS].T.astype(BF16_NP)  # [1024, 512]
        xs_l = np.empty((P, KO * CW), BF16_NP)
        for k in range(KO):
            rows = slice(k * P, (k + 1) * P)
            off = k * CW
            xs_l[:, off : off + BS] = xT[rows]
            xs_l[:, off + BS : off + CW] = Sb[rows]
        in_maps.append({"xs": xs_l, "bs": b_l})

    nc = _get_nc()
    res = run_bass_kernel_spmd(
        nc,
        in_maps,
        core_ids=list(range(NCORES)),
        trace=bool(int(os.environ.get("KERNEL_TRACE", "0"))),
    )
    LAST_RESULTS = res

    outs = []
    for c in range(NCORES):
        o = np.asarray(res.results[c]["o"], dtype=np.float32)
        outT = o.reshape(P, MO, BS).transpose(1, 0, 2).reshape(D, BS)
        outs.append(outT.T)
    return np.ascontiguousarray(np.concatenate(outs, axis=0))
